# revision 1
# baseline (speedup 1.0000x reference)
"""DeepseekMoE (B=4, S=2048, D=2048, E=64 top-6 + shared expert) on 8 trn2 NeuronCores.

Strategy (expert-parallel, per the sharding hint):
  - Host computes the router (softmax top-6, renorm) in fp32 exactly as the
    reference, sorts assignments by expert, and packs each expert's routed
    tokens into a transposed bf16 activation buffer.
  - The 64 experts are distributed 8-per-core, bin-packed by routed count so
    every core runs an identical static program (slot capacities are the max
    count in each rank octet, rounded up to 128).
  - The shared expert is computed on-device as two extra "expert" slots per
    core (the FS=2816 hidden dim split in half), data-parallel over tokens.
  - Each core runs grouped SwiGLU GEMMs (bf16 inputs, fp32 PSUM accumulate)
    over its slots and applies the combine weights on-chip.
  - Host scatters the weighted expert rows back per assignment and reduces
    the 6 contributions per token, then adds the shared-expert output.

All heavy FLOPs (expert GEMMs, SwiGLU, shared expert, combine scaling) run on
the NeuronCores; the host does only routing index math and data packing.
"""

import math
import os

import numpy as np
import ml_dtypes
from einops import rearrange

import concourse.bacc as bacc
import concourse.mybir as mybir
import concourse.tile as tile
from concourse.bass_utils import run_bass_kernel_spmd

BF16 = mybir.dt.bfloat16
F32 = mybir.dt.float32
NPBF16 = ml_dtypes.bfloat16

TOP_K = 6
CAP_FACTOR = 2
N_CORES = 8
CHUNK = 512  # matmul moving-operand free-dim / PSUM bank width (fp32)
P = 128


def _chunks(total, step):
    return [(o, min(step, total - o)) for o in range(0, total, step)]


class _Cfg:
    """Static per-program configuration (identical across the 8 cores)."""

    def __init__(self, caps_routed, D, F, TS):
        assert D % P == 0 and F % P == 0
        self.D, self.F, self.TS = D, F, TS
        self.KD = D // P  # contraction subtiles for D
        self.NF = F // P  # f tiles (also contraction subtiles for F in down)
        self.caps = list(caps_routed) + [TS, TS]  # slots: 8 routed + 2 shared halves
        self.nslots = len(self.caps)
        self.Rtot = int(sum(caps_routed))
        self.RX = self.Rtot + TS  # x columns (shared slots alias the same slice)
        self.RY = self.Rtot + 2 * TS  # output rows (shared halves separate)
        assert self.RX % P == 0 and self.RY % P == 0
        # x-column offset and y-row offset per slot
        off = np.concatenate([[0], np.cumsum(caps_routed)]).astype(int)
        self.xoff = list(off[:-1]) + [self.Rtot, self.Rtot]
        self.yoff = list(off[:-1]) + [self.Rtot, self.Rtot + TS]

    def key(self):
        return (tuple(self.caps), self.D, self.F, self.TS)


def _emit(nc, cfg):
    """Emit the per-core Tile program. Expects dram tensors declared on nc."""
    KD, NF, D = cfg.KD, cfg.NF, cfg.D
    xpt = nc.dram_tensor("xpt", [P, KD, cfg.RX], BF16, kind="ExternalInput")
    wg = nc.dram_tensor("wg", [cfg.nslots, NF, P, KD, P], BF16, kind="ExternalInput")
    wu = nc.dram_tensor("wu", [cfg.nslots, NF, P, KD, P], BF16, kind="ExternalInput")
    wd = nc.dram_tensor("wd", [cfg.nslots, P, NF, D], BF16, kind="ExternalInput")
    wv = nc.dram_tensor("wv", [P, cfg.RY // P], F32, kind="ExternalInput")
    yp = nc.dram_tensor("yp", [cfg.RY, D], F32, kind="ExternalOutput")

    with tile.TileContext(nc) as tc:
        with (
            tc.tile_pool(name="xp", bufs=3) as xp_pool,
            tc.tile_pool(name="wgu", bufs=3) as wgu_pool,
            tc.tile_pool(name="wdp", bufs=1) as wd_pool,
            tc.tile_pool(name="htp", bufs=2) as ht_pool,
            tc.tile_pool(name="silp", bufs=2) as sil_pool,
            tc.tile_pool(name="yop", bufs=4) as yo_pool,
            tc.tile_pool(name="wvp", bufs=1) as wv_pool,
            tc.tile_pool(name="pgp", bufs=2, space="PSUM") as pg_pool,
            tc.tile_pool(name="pup", bufs=2, space="PSUM") as pu_pool,
            tc.tile_pool(name="pop", bufs=2, space="PSUM") as po_pool,
        ):
            wv_sb = wv_pool.tile([P, cfg.RY // P], F32, tag="wv")
            nc.sync.dma_start(wv_sb[:], wv[:])

            for s in range(cfg.nslots):
                cap = cfg.caps[s]
                xo, yo_row = cfg.xoff[s], cfg.yoff[s]
                # down-proj weights for this slot, resident across row chunks
                wdsb = wd_pool.tile([P, NF, D], BF16, tag="wd")
                nc.sync.dma_start(wdsb[:], wd[s])

                for ro, w in _chunks(cap, CHUNK):
                    xs = xp_pool.tile([P, KD, CHUNK], BF16, tag="xs")
                    nc.sync.dma_start(xs[:, :, :w], xpt[:, :, xo + ro : xo + ro + w])
                    ht = ht_pool.tile([P, NF, CHUNK], BF16, tag="ht")

                    for ft in range(NF):
                        wgs = wgu_pool.tile([P, KD, P], BF16, tag="wg")
                        nc.sync.dma_start(wgs[:], wg[s, ft])
                        wus = wgu_pool.tile([P, KD, P], BF16, tag="wu")
                        nc.sync.dma_start(wus[:], wu[s, ft])
                        pg = pg_pool.tile([P, CHUNK], F32, tag="pg")
                        pu = pu_pool.tile([P, CHUNK], F32, tag="pu")
                        for ks in range(KD):
                            nc.tensor.matmul(
                                pg[:, :w], wgs[:, ks], xs[:, ks, :w],
                                start=(ks == 0), stop=(ks == KD - 1),
                            )
                        for ks in range(KD):
                            nc.tensor.matmul(
                                pu[:, :w], wus[:, ks], xs[:, ks, :w],
                                start=(ks == 0), stop=(ks == KD - 1),
                            )
                        sil = sil_pool.tile([P, CHUNK], F32, tag="sil")
                        nc.scalar.activation(
                            sil[:, :w], pg[:, :w], mybir.ActivationFunctionType.Silu
                        )
                        nc.vector.tensor_tensor(
                            ht[:, ft, :w], sil[:, :w], pu[:, :w], mybir.AluOpType.mult
                        )

                    # down projection for this row chunk
                    for mc in range(w // P):
                        grow = yo_row + ro + mc * P  # global output row (multiple of P)
                        for do, dw in _chunks(D, CHUNK):
                            po = po_pool.tile([P, CHUNK], F32, tag="po")
                            for kf in range(NF):
                                nc.tensor.matmul(
                                    po[:, :dw],
                                    ht[:, kf, mc * P : (mc + 1) * P],
                                    wdsb[:, kf, do : do + dw],
                                    start=(kf == 0), stop=(kf == NF - 1),
                                )
                            yot = yo_pool.tile([P, CHUNK], F32, tag="yo")
                            nc.vector.tensor_scalar_mul(
                                yot[:, :dw], po[:, :dw],
                                wv_sb[:, grow // P : grow // P + 1],
                            )
                            nc.sync.dma_start(
                                yp[grow : grow + P, do : do + dw], yot[:, :dw]
                            )
    nc.compile()
    return nc


_PROGRAM_CACHE: dict = {}


def _build_program(cfg):
    key = cfg.key()
    if key not in _PROGRAM_CACHE:
        nc = bacc.Bacc("TRN2", target_bir_lowering=False, debug=False,
                       enable_asserts=False)
        _PROGRAM_CACHE[key] = _emit(nc, cfg)
    return _PROGRAM_CACHE[key]


# ---------------------------------------------------------------- host routing


def _route(x32, gate_w):
    """Mirror the reference router; returns per-assignment (sorted by expert)."""
    N, D = x32.shape
    E = gate_w.shape[0]
    A = N * TOP_K
    C = CAP_FACTOR * ((A + E - 1) // E)
    topw = topi = None
    try:
        import jax
        import jax.numpy as jnp

        with jax.default_device(jax.devices("cpu")[0]):
            scores = jax.nn.softmax(jnp.asarray(x32) @ jnp.asarray(gate_w).T, axis=-1)
            tw, ti = jax.lax.top_k(scores, TOP_K)
            tw = tw / (tw.sum(-1, keepdims=True) + 1e-20)
            topw = np.asarray(tw)
            topi = np.asarray(ti)
    except Exception:
        logits = x32 @ gate_w.T
        logits -= logits.max(-1, keepdims=True)
        ex = np.exp(logits)
        scores = ex / ex.sum(-1, keepdims=True)
        topi = np.argsort(-scores, axis=-1, kind="stable")[:, :TOP_K]
        tw = np.take_along_axis(scores, topi, axis=-1)
        topw = tw / (tw.sum(-1, keepdims=True) + 1e-20)

    flat_e = topi.reshape(-1).astype(np.int64)
    flat_w = topw.reshape(-1).astype(np.float32)
    order = np.argsort(flat_e, kind="stable")
    st = order // TOP_K  # token index per sorted assignment
    sw = flat_w[order]
    counts = np.bincount(flat_e, minlength=E)
    offs = np.cumsum(counts) - counts
    eff = np.minimum(counts, C)  # capacity-dropped tail (never hit in practice)
    return st, sw, counts.astype(int), offs.astype(int), eff.astype(int)


def _plan(eff):
    """Assign experts to (core, slot) so all cores share one static program."""
    E = len(eff)
    nslots = E // N_CORES
    rank = np.argsort(-eff, kind="stable")
    expert_of = np.zeros((N_CORES, nslots), dtype=int)
    caps = []
    for s in range(nslots):
        octet = rank[s * N_CORES : (s + 1) * N_CORES]
        expert_of[:, s] = octet
        caps.append(max(P, int(math.ceil(eff[octet].max() / P)) * P))
    return expert_of, caps


def _chunk_gate(mat16):
    # [F, D] -> [NF, P(d), KD, P(f)] matmul-ready lhsT chunks
    return np.ascontiguousarray(
        rearrange(mat16, "(ft f) (ks p) -> ft p ks f", f=P, p=P)
    )


def _chunk_down(mat16):
    # [D, F] -> [P(f), KF, D] rhs slabs
    return np.ascontiguousarray(rearrange(mat16, "d (kf p) -> p kf d", p=P))


def prepare(inputs):
    """Host routing + packing. Returns everything needed to run + combine."""
    hidden_states = np.asarray(inputs["hidden_states"], dtype=np.float32)
    gate_w = np.asarray(inputs["gate_w"], dtype=np.float32)
    w_gate = np.asarray(inputs["w_gate"])
    w_up = np.asarray(inputs["w_up"])
    w_down = np.asarray(inputs["w_down"])
    sh_gate = np.asarray(inputs["sh_gate"])
    sh_up = np.asarray(inputs["sh_up"])
    sh_down = np.asarray(inputs["sh_down"])

    B, S, D = hidden_states.shape
    E, F, _ = w_gate.shape
    FS = sh_gate.shape[0]
    assert FS == 2 * F, "shared expert hidden dim must be 2x expert hidden dim"
    N = B * S
    TS = N // N_CORES
    x32 = hidden_states.reshape(N, D)

    st, sw, counts, offs, eff = _route(x32, gate_w)
    expert_of, caps = _plan(eff)
    cfg = _Cfg(caps, D, F, TS)

    # transposed bf16 activations: xr[p, ks, n] = x[n, ks*P + p]
    xb = x32.astype(NPBF16)
    xr = np.ascontiguousarray(xb.T.reshape(cfg.KD, P, N).transpose(1, 0, 2))

    # per-expert chunked weights (bf16)
    g16 = w_gate.astype(NPBF16)
    u16 = w_up.astype(NPBF16)
    d16 = w_down.astype(NPBF16)
    shg = [_chunk_gate(sh_gate[h * F : (h + 1) * F].astype(NPBF16)) for h in (0, 1)]
    shu = [_chunk_gate(sh_up[h * F : (h + 1) * F].astype(NPBF16)) for h in (0, 1)]
    shd = [_chunk_down(sh_down[:, h * F : (h + 1) * F].astype(NPBF16)) for h in (0, 1)]

    in_maps = []
    for c in range(N_CORES):
        xpt = np.zeros((P, cfg.KD, cfg.RX), dtype=NPBF16)
        wvf = np.zeros(cfg.RY, dtype=np.float32)
        wgs, wus, wds = [], [], []
        for s in range(8):
            e = expert_of[c, s]
            cnt = eff[e]
            toks = st[offs[e] : offs[e] + cnt]
            xpt[:, :, cfg.xoff[s] : cfg.xoff[s] + cnt] = xr[:, :, toks]
            wvf[cfg.yoff[s] : cfg.yoff[s] + cnt] = sw[offs[e] : offs[e] + cnt]
            wgs.append(_chunk_gate(g16[e]))
            wus.append(_chunk_gate(u16[e]))
            wds.append(_chunk_down(d16[e]))
        xpt[:, :, cfg.Rtot : cfg.RX] = xr[:, :, c * TS : (c + 1) * TS]
        wvf[cfg.Rtot :] = 1.0
        wgs += shg
        wus += shu
        wds += shd
        in_maps.append(
            {
                "xpt": xpt,
                "wg": np.stack(wgs),
                "wu": np.stack(wus),
                "wd": np.stack(wds),
                "wv": np.ascontiguousarray(
                    wvf.reshape(cfg.RY // P, P).T
                ).astype(np.float32),
            }
        )

    meta = dict(
        cfg=cfg, st=st, offs=offs, eff=eff, expert_of=expert_of,
        N=N, D=D, B=B, S=S, TS=TS,
    )
    return cfg, in_maps, meta


def combine(meta, per_core_yp):
    """Scatter weighted expert rows back to tokens and add the shared expert."""
    cfg = meta["cfg"]
    N, D, TS = meta["N"], meta["D"], meta["TS"]
    st, offs, eff, expert_of = meta["st"], meta["offs"], meta["eff"], meta["expert_of"]
    A = N * TOP_K
    zfull = np.zeros((A, D), dtype=np.float32)
    for c in range(N_CORES):
        ypc = per_core_yp[c]
        for s in range(8):
            e = expert_of[c, s]
            cnt = eff[e]
            zfull[offs[e] : offs[e] + cnt] = ypc[cfg.yoff[s] : cfg.yoff[s] + cnt]
    order2 = np.argsort(st, kind="stable")
    y = zfull[order2].reshape(N, TOP_K, D).sum(axis=1)
    for c in range(N_CORES):
        ypc = per_core_yp[c]
        y[c * TS : (c + 1) * TS] += ypc[cfg.Rtot : cfg.Rtot + TS]
        y[c * TS : (c + 1) * TS] += ypc[cfg.Rtot + TS : cfg.RY]
    return y.reshape(meta["B"], meta["S"], D).astype(np.float32)


def kernel(**inputs) -> np.ndarray:
    cfg, in_maps, meta = prepare(inputs)
    nc = _build_program(cfg)
    res = run_bass_kernel_spmd(nc, in_maps, list(range(N_CORES)))
    return combine(meta, [r["yp"] for r in res.results])
